# revision 21
# baseline (speedup 1.0000x reference)
"""Trainium2 Bass kernel for batched cross-attention (CoupletsAttentionModel).

Reference computation (per batch element b):
    S = dec @ enc^T          [S_dec, S_enc]
    P = softmax(S, axis=-1)
    O = P @ enc              [S_dec, D]

Sharding: data-parallel over batch — B=8 batch elements, one per NeuronCore.
Each core runs an identical (SPMD) program on its own batch slice; no
collectives, host stacks the 8 per-core outputs.

Per-core algorithm (S_enc=S_dec=2048, D=512, fp32 in/out), final:
  - fp16 matmuls; PE streams 512-col fp16 matmuls at ~215.6ns when the
    power governor allows full clock. NOTE: sustained near-100% PE duty
    trips a DVFS throttle (~1.2x slowdown) whose state is sticky and
    seeded by early-kernel activity density — the prologue structure
    below was tuned empirically to keep the whole run at full clock.
  - Fat-descriptor input DMA: HBM tensors viewed as [128, 16*512] so
    each partition receives 16 consecutive rows (4-8KB contiguous
    descriptors instead of 2KB). Induces a global row permutation
    k = 16*p + j applied consistently to V slots, enc^T free order,
    P^T partitions and the q/output mapping — softmax is permutation-
    invariant across k, so results are exact.
  - enc split across both HWDGE queues; dec trails on sync. enc^T via
    PE transposes in 2-tile pair groups ([128,8,128] PSUM pair-copies
    split across DVE/ScalarE); dec^T (qT, layout [128, j, dt, 128])
    via PE transposes, one tile per loop iteration.
  - Main loop per 128-row q-tile: S in 2 PSUM chunks of 1024; partial
    row-max reduces (per 512) overlap the S matmuls; per-chunk exp with
    own max (ScalarE, accum sums); ONE full-row P^T xbar transpose-DMA
    per tile (ScalarE queue, serialized chain — concurrent xbar
    transposes corrupt); per-chunk O accumulation in PSUM; final
    O = g0*O_c0 + g1*O_c1 with g_c = exp(m_c - m_g)/Z (exact), via DVE
    muls + gpsimd add (gpsimd cannot touch PSUM).
  - PE software pipelining: O(t-1) matmuls issue after S(t), giving the
    softmax+transpose chain a full S-block of slack. The LAST tile's
    P^T runs on the (then idle) PE instead of the xbar to dodge the
    tail downclock.
  - PSUM: s-chunks 2 banks x bufs=2; shared tag "po" (warmup bursts,
    transpose staging, O accumulators) 1 bank x bufs=4 -> exactly 8.
  - HAM warmup bursts keep the PE clock ramping during the load phase.
"""

import contextlib
import ctypes
import os
import sys
import types

import numpy as np

import concourse.bass as bass
import concourse.tile as tile
from concourse import bacc, mybir
from concourse import bass_utils
from concourse.masks import make_identity

F32 = mybir.dt.float32
F16 = mybir.dt.float16
AX = mybir.AxisListType
AFT = mybir.ActivationFunctionType

N_CORES = 8
PART = 128


def attention_tile_kernel(tc, out_ap, dec_ap, enc_ap, seq, d):
    nc = tc.nc
    P = PART
    KC = 512           # matmul moving free dim / one fp32 PSUM bank
    SC = 1024          # softmax chunk (2 per row)
    n_qt = seq // P    # 16
    n_dt = d // P      # 4
    n_sc = seq // SC   # 2
    jps = SC // P      # j-slots per chunk (8)
    n_hf = SC // KC    # 512-halves per chunk (2)

    enc_flat = enc_ap.rearrange("(p j) d -> p (j d)", p=P)
    dec_flat = dec_ap.rearrange("(p j) d -> p (j d)", p=P)
    out_view = out_ap.rearrange("(p j) d -> p j d", p=P)

    stack = contextlib.ExitStack()
    pool = lambda **kw: stack.enter_context(tc.tile_pool(**kw))

    singles = pool(name="singles", bufs=1)
    big = pool(name="big", bufs=1)
    stage = pool(name="stage", bufs=3)
    d16p = pool(name="d16p", bufs=3)
    psum = pool(name="psum", bufs=1, space="PSUM")
    p_pool = pool(name="p_pool", bufs=2)
    pt_pool = pool(name="pt_pool", bufs=3)
    stats = pool(name="stats", bufs=3)
    osb = pool(name="osb", bufs=2)

    with stack:
        ident = singles.tile([P, P], F16)
        make_identity(nc, ident[:])
        dummy = singles.tile([P, KC], F16)
        nc.vector.memset(dummy[:], 0.25)

        v_flat = big.tile([P, seq * d // P], F16)  # enc f16, slot j = [:, j*d:...]
        kT = big.tile([P, n_dt, seq], F16)         # enc^T  [d_in, dt, kfree]
        qT = big.tile([P, n_qt, n_dt, P], F16)     # dec^T  [d_in, j, dt, q]

        def warm_burst(n, name):
            wp = psum.tile([P, KC], F32, tag="po", bufs=4, name=name)
            for i in range(n):
                nc.tensor.matmul(
                    wp[:], dummy[:, 0:P], dummy[:], start=(i == 0), stop=(i == n - 1)
                )

        warm_burst(12, "warm0")

        # ---- input loads (fat descriptors) ----
        EQ = 2048  # enc quarter (f32 elems per partition, covers 4 j-slots)
        DC = 1024  # dec chunk (2 j-slots)
        enc32 = {}
        dec32 = {}

        def load(eng, src, width, lo, name):
            t = stage.tile([P, width], F32, tag="ld32", bufs=6, name=name)
            eng.dma_start(out=t[:], in_=src[:, lo : lo + width])
            return t

        # sync: enc E0, E1 then all of dec; scalar: dec c0 then enc E2, E3
        dec32[0] = load(nc.scalar, dec_flat, DC, 0, "d32_0")
        enc32[0] = load(nc.sync, enc_flat, EQ, 0, "e32_0")
        enc32[2] = load(nc.scalar, enc_flat, EQ, 2 * EQ, "e32_2")
        enc32[1] = load(nc.sync, enc_flat, EQ, EQ, "e32_1")
        enc32[3] = load(nc.scalar, enc_flat, EQ, 3 * EQ, "e32_3")
        for c in range(1, 8):
            dec32[c] = load(nc.sync, dec_flat, DC, c * DC, f"d32_{c}")

        # ---- enc^T via PE transposes in 2-tile pair groups; pair copy
        # [128,8,128] PSUM->SBUF on DVE (j0-7) / ScalarE (j8-15)
        def prep_kT_pair(jp, eng):
            tps = psum.tile(
                [P, n_dt, 2, P], F16, tag="po", bufs=4, name=f"tps_{jp}"
            )
            for hh in range(2):
                j = 2 * jp + hh
                for dc in range(n_dt):
                    nc.tensor.transpose(
                        tps[:, dc, hh, :],
                        v_flat[:, j * d + dc * P : j * d + (dc + 1) * P],
                        ident[:],
                    )
            dst = kT[:, :, 2 * jp * P : (2 * jp + 2) * P].rearrange(
                "p dt (hh b) -> p dt hh b", hh=2
            )
            if eng is nc.vector:
                eng.tensor_copy(dst, tps[:])
            else:
                eng.activation(dst, tps[:], AFT.Copy)

        def cast_enc(e):
            dst = v_flat[:, e * EQ : (e + 1) * EQ]
            if e in (0, 1):
                nc.vector.tensor_copy(dst, enc32[e][:])
            else:
                nc.scalar.activation(dst, enc32[e][:], AFT.Copy)

        cast_enc(0)          # DVE, j0-3
        cast_enc(2)          # ScalarE, j8-11
        prep_kT_pair(0, nc.vector)
        prep_kT_pair(1, nc.vector)
        warm_burst(6, "warm1")
        prep_kT_pair(4, nc.scalar)
        prep_kT_pair(5, nc.scalar)
        cast_enc(1)          # DVE, j4-7
        cast_enc(3)          # ScalarE, j12-15
        warm_burst(6, "warm2")
        prep_kT_pair(2, nc.vector)
        prep_kT_pair(3, nc.vector)
        warm_burst(6, "warm3")
        prep_kT_pair(6, nc.scalar)
        prep_kT_pair(7, nc.scalar)

        # ---- dec casts (DVE); qT tiles 0-3 via PE transposes in the
        # prologue, later tiles ride the xbar chain one per iteration
        d16 = {}

        def cast_dec(c):
            t = d16p.tile([P, DC], F16, tag="d16", name=f"d16_{c}")
            nc.vector.tensor_copy(t[:], dec32[c][:])
            d16[c] = t

        prev_t = [None]

        def xbar(out_tile, in_view, eng=None):
            ti = (eng or nc.scalar).dma_start(
                out=out_tile, in_=in_view, transpose=True
            )
            if prev_t[0] is not None:
                tile.add_dep_helper(ti.ins, prev_t[0].ins, reason="serialize xbar")
            prev_t[0] = ti

        def prep_qT_pe(j):
            c, h = j // 2, j % 2
            tps = psum.tile([P, n_dt, P], F16, tag="po", bufs=4, name=f"tq_{j}")
            for dc in range(n_dt):
                nc.tensor.transpose(
                    tps[:, dc, :], d16[c][:, h * d + dc * P : h * d + (dc + 1) * P],
                    ident[:],
                )
            nc.vector.tensor_copy(qT[:, j, :, :], tps[:])

        cast_dec(0)
        prep_qT_pe(0)
        prep_qT_pe(1)

        # ---- main loop ----
        pending = [None]
        PACE_W = 0  # f16 elems; tunes the per-tile PE idle gap (DVFS duty)
        pace_sb = (
            singles.tile([P, PACE_W], F16, name="pace_sb") if PACE_W else None
        )

        def emit_O(st, pacer):
            qt, pt, g = st
            o_cs = []
            for c in range(n_sc):
                o_c = psum.tile([P, d], F32, tag="po", bufs=4, name=f"o_{qt}_{c}")
                for jj in range(jps):
                    mi = nc.tensor.matmul(
                        o_c[:],
                        pt[:, c * jps + jj, :],
                        v_flat[:, (c * jps + jj) * d : (c * jps + jj + 1) * d],
                        start=(jj == 0),
                        stop=(jj == jps - 1),
                    )
                    if c == 0 and jj == 0 and pacer is not None:
                        tile.add_dep_helper(mi.ins, pacer.ins, reason="pace PE")
                o_cs.append(o_c)
            o_s0 = osb.tile([P, d], F32, tag="osb0", name=f"osb0_{qt}")
            o_s1 = osb.tile([P, d], F32, tag="osb1", name=f"osb1_{qt}")
            nc.vector.tensor_scalar_mul(o_s0[:], o_cs[0][:], g[:, 0:1])
            nc.vector.tensor_scalar_mul(o_s1[:], o_cs[1][:], g[:, 1:2])
            o_f = osb.tile([P, d], F32, tag="osbf", name=f"osbf_{qt}")
            nc.gpsimd.tensor_tensor(
                o_f[:], o_s0[:], o_s1[:], op=mybir.AluOpType.add
            )
            nc.sync.dma_start(out=out_view[:, qt, :], in_=o_f[:])

        for qt in range(n_qt):
            q0 = qt * P
            # S matmuls chunk-major; partial row-max reduces interleaved
            nmx4 = stats.tile([P, n_sc * n_hf], F32, tag="nmx4")
            s_cs = []
            for c in range(n_sc):
                s_c = psum.tile([P, SC], F32, tag="s", bufs=2, name=f"s_{qt}_{c}")
                for h in range(n_hf):
                    k0 = c * SC + h * KC
                    for dt_ in range(n_dt):
                        nc.tensor.matmul(
                            s_c[:, h * KC : (h + 1) * KC],
                            qT[:, qt, dt_, :],
                            kT[:, dt_, k0 : k0 + KC],
                            start=(dt_ == 0),
                            stop=(dt_ == n_dt - 1),
                        )
                    nc.vector.tensor_reduce(
                        nmx4[:, c * n_hf + h : c * n_hf + h + 1],
                        s_c[:, h * KC : (h + 1) * KC],
                        axis=AX.X, op=mybir.AluOpType.max, negate=True,
                    )
                s_cs.append(s_c)

            # per-chunk softmax (exp with own max) + one full-row P^T
            nmx = stats.tile([P, n_sc], F32, tag="nmx")
            sums = stats.tile([P, n_sc], F32, tag="sums")
            p_sb = p_pool.tile([P, seq], F16)
            pacer = None
            for c in range(n_sc):
                nc.vector.tensor_reduce(
                    nmx[:, c : c + 1], nmx4[:, c * n_hf : (c + 1) * n_hf],
                    axis=AX.X, op=mybir.AluOpType.min,
                )
                if c == n_sc - 1 and pace_sb is not None:
                    pacer = nc.vector.memset(pace_sb[:], 0.5)
                nc.scalar.activation(
                    p_sb[:, c * SC : (c + 1) * SC],
                    s_cs[c][:],
                    AFT.Exp,
                    bias=nmx[:, c : c + 1],
                    scale=1.0,
                    accum_out=sums[:, c : c + 1],
                )
            pt = pt_pool.tile([P, seq // P, P], F16, tag="pt", name=f"pt_{qt}")
            if qt < n_qt - 1:
                xbar(pt[:], p_sb[:])

            # global correction: g_c = exp(m_c - m_g) / Z
            mxp = stats.tile([P, n_sc], F32, tag="mxp")
            nc.vector.tensor_scalar_mul(mxp[:], nmx[:], -1.0)
            negmg = stats.tile([P, 1], F32, tag="negmg")
            nc.vector.tensor_reduce(
                negmg[:], nmx4[:], axis=AX.X, op=mybir.AluOpType.min
            )
            fsc = stats.tile([P, n_sc], F32, tag="fsc")
            nc.scalar.activation(fsc[:], mxp[:], AFT.Exp, bias=negmg[:], scale=1.0)
            ssc = stats.tile([P, n_sc], F32, tag="ssc")
            nc.vector.tensor_mul(ssc[:], sums[:], fsc[:])
            sm = stats.tile([P, 1], F32, tag="sm")
            nc.vector.reduce_sum(sm[:], ssc[:], axis=AX.X)
            rinv = stats.tile([P, 1], F32, tag="rinv")
            nc.vector.reciprocal(rinv[:], sm[:])
            g = stats.tile([P, n_sc], F32, tag="g")
            nc.vector.tensor_scalar_mul(g[:], fsc[:], rinv[:])

            if pending[0] is not None:
                emit_O(pending[0], pacer)
            if qt == n_qt - 1:
                # last tile: P^T on the now-idle PE (the xbar chain latency
                # plus the tail downclock would cost ~5us here)
                for grp in range(4):
                    tq = psum.tile(
                        [P, 4, P], F16, tag="po", bufs=4, name=f"ptpe_{grp}"
                    )
                    for k2 in range(4):
                        j = grp * 4 + k2
                        nc.tensor.transpose(
                            tq[:, k2, :], p_sb[:, j * P : (j + 1) * P], ident[:]
                        )
                    nc.vector.tensor_copy(pt[:, grp * 4 : (grp + 1) * 4, :], tq[:])
            pending[0] = (qt, pt, g)

            # dec chunk casts at DVE tail (every other iter); qT for tile
            # qt+4 via PE transposes at the block tail (~0.45us, keeps PE
            # duty under the DVFS activity threshold)
            if qt == 0:
                cast_dec(1)
                prep_qT_pe(2)
                prep_qT_pe(3)
            if qt % 2 == 0 and qt // 2 + 2 < (seq * d // P) // DC:
                cast_dec(qt // 2 + 2)
            if qt + 4 < n_qt:
                prep_qT_pe(qt + 4)

        emit_O(pending[0], None)


def build(seq=2048, d=512, n_cores=N_CORES):
    nc = bacc.Bacc(
        "TRN2", target_bir_lowering=False, debug=False, num_devices=n_cores
    )
    dec = nc.dram_tensor("dec", [seq, d], F32, kind="ExternalInput").ap()
    enc = nc.dram_tensor("enc", [seq, d], F32, kind="ExternalInput").ap()
    out = nc.dram_tensor("out", [seq, d], F32, kind="ExternalOutput").ap()
    with tile.TileContext(nc) as tc:
        attention_tile_kernel(tc, out, dec, enc, seq, d)
    nc.compile()
    return nc


# ---------------------------------------------------------------------------
# Optional NTFF profiling support (used by our own test harness; inert unless
# BASSKERNEL_TRACE=1). The agent image lacks `antenv.axon_hooks`, so recreate
# it in sys.modules with a ctypes hook against libaxon_pjrt.so.
# ---------------------------------------------------------------------------
LAST_EXEC_TIME_NS = None


def _install_profile_hook():
    so_path = "/opt/axon/libaxon_pjrt.so"
    if "antenv.axon_hooks" in sys.modules or not os.path.exists(so_path):
        return
    lib = ctypes.CDLL(so_path)
    if not hasattr(lib, "axon_start_nrt_profile"):
        return
    lib.axon_start_nrt_profile.argtypes = [
        ctypes.POINTER(ctypes.c_int64),
        ctypes.c_size_t,
    ]
    lib.axon_start_nrt_profile.restype = ctypes.c_int64
    lib.axon_stop_nrt_profile.argtypes = [ctypes.c_char_p]
    lib.axon_stop_nrt_profile.restype = ctypes.c_int64

    @contextlib.contextmanager
    def _hook(output_dir, device_ids):
        import jax

        jax.devices()
        if device_ids:
            ids = (ctypes.c_int64 * len(device_ids))(*device_ids)
            rc = lib.axon_start_nrt_profile(ids, len(device_ids))
        else:
            rc = lib.axon_start_nrt_profile(None, 0)
        if rc != 0:
            raise RuntimeError(f"axon_start_nrt_profile rc={rc}")
        try:
            yield
        finally:
            n = lib.axon_stop_nrt_profile(str(output_dir).encode())
            print(f"ntff profile: {n} file(s) written to {output_dir}")

    mod = types.ModuleType("antenv.axon_hooks")
    _state = {"hook": _hook}
    mod.set_axon_ntff_profile_hook = lambda h: _state.__setitem__("hook", h)
    mod.get_axon_ntff_profile_hook = lambda: _state["hook"]
    sys.modules["antenv.axon_hooks"] = mod
    bass_utils.upload_artifacts = lambda tmpdir: tmpdir


_NC_CACHE = {}


def kernel(enc_outputs: np.ndarray, dec_outputs: np.ndarray) -> np.ndarray:
    B, seq, d = dec_outputs.shape
    assert enc_outputs.shape == (B, seq, d) and B == N_CORES

    trace = os.environ.get("BASSKERNEL_TRACE", "0") == "1"
    if trace:
        _install_profile_hook()

    key = (seq, d)
    if key not in _NC_CACHE:
        _NC_CACHE[key] = build(seq, d)
    nc = _NC_CACHE[key]

    in_maps = [
        {
            "dec": np.ascontiguousarray(dec_outputs[b], dtype=np.float32),
            "enc": np.ascontiguousarray(enc_outputs[b], dtype=np.float32),
        }
        for b in range(B)
    ]
    res = bass_utils.run_bass_kernel_spmd(
        nc,
        in_maps,
        core_ids=list(range(N_CORES)),
        trace=trace,
        tmpdir=os.environ.get("BASSKERNEL_TRACE_DIR") if trace else None,
    )
    global LAST_EXEC_TIME_NS
    LAST_EXEC_TIME_NS = res.exec_time_ns
    out = np.stack([res.results[b]["out"] for b in range(B)], axis=0)
    return out.astype(np.float32)


# revision 22
# speedup vs baseline: 1.0088x; 1.0088x over previous
"""Trainium2 Bass kernel for batched cross-attention (CoupletsAttentionModel).

Reference computation (per batch element b):
    S = dec @ enc^T          [S_dec, S_enc]
    P = softmax(S, axis=-1)
    O = P @ enc              [S_dec, D]

Sharding: data-parallel over batch — B=8 batch elements, one per NeuronCore.
Each core runs an identical (SPMD) program on its own batch slice; no
collectives, host stacks the 8 per-core outputs.

Per-core algorithm (S_enc=S_dec=2048, D=512, fp32 in/out), final:
  - fp16 matmuls; PE streams 512-col fp16 matmuls at ~215.6ns when the
    power governor allows full clock. NOTE: sustained near-100% PE duty
    trips a DVFS throttle (~1.2x slowdown) whose state is sticky and
    seeded by early-kernel activity density — the prologue structure
    below was tuned empirically to keep the whole run at full clock.
  - Fat-descriptor input DMA: HBM tensors viewed as [128, 16*512] so
    each partition receives 16 consecutive rows (4-8KB contiguous
    descriptors instead of 2KB). Induces a global row permutation
    k = 16*p + j applied consistently to V slots, enc^T free order,
    P^T partitions and the q/output mapping — softmax is permutation-
    invariant across k, so results are exact.
  - enc split across both HWDGE queues; dec trails on sync. enc^T via
    PE transposes in 2-tile pair groups ([128,8,128] PSUM pair-copies
    split across DVE/ScalarE); dec^T (qT, layout [128, j, dt, 128])
    via PE transposes, one tile per loop iteration.
  - Main loop per 128-row q-tile: S in 2 PSUM chunks of 1024; partial
    row-max reduces (per 512) overlap the S matmuls; per-chunk exp with
    own max (ScalarE, accum sums); ONE full-row P^T xbar transpose-DMA
    per tile (ScalarE queue, serialized chain — concurrent xbar
    transposes corrupt); per-chunk O accumulation in PSUM; final
    O = g0*O_c0 + g1*O_c1 with g_c = exp(m_c - m_g)/Z (exact), via DVE
    muls + gpsimd add (gpsimd cannot touch PSUM).
  - PE software pipelining: O(t-1) matmuls issue after S(t), giving the
    softmax+transpose chain a full S-block of slack. The LAST tile's
    P^T runs on the (then idle) PE instead of the xbar to dodge the
    tail downclock.
  - PSUM: s-chunks 2 banks x bufs=2; shared tag "po" (warmup bursts,
    transpose staging, O accumulators) 1 bank x bufs=4 -> exactly 8.
  - HAM warmup bursts keep the PE clock ramping during the load phase.
"""

import contextlib
import ctypes
import os
import sys
import types

import numpy as np

import concourse.bass as bass
import concourse.tile as tile
from concourse import bacc, mybir
from concourse import bass_utils
from concourse.masks import make_identity

F32 = mybir.dt.float32
F16 = mybir.dt.float16
AX = mybir.AxisListType
AFT = mybir.ActivationFunctionType

N_CORES = 8
PART = 128


def attention_tile_kernel(tc, out_ap, dec_ap, enc_ap, seq, d):
    nc = tc.nc
    P = PART
    KC = 512           # matmul moving free dim / one fp32 PSUM bank
    SC = 1024          # softmax chunk (2 per row)
    n_qt = seq // P    # 16
    n_dt = d // P      # 4
    n_sc = seq // SC   # 2
    jps = SC // P      # j-slots per chunk (8)
    n_hf = SC // KC    # 512-halves per chunk (2)

    enc_flat = enc_ap.rearrange("(p j) d -> p (j d)", p=P)
    dec_flat = dec_ap.rearrange("(p j) d -> p (j d)", p=P)
    out_view = out_ap.rearrange("(p j) d -> p j d", p=P)

    stack = contextlib.ExitStack()
    pool = lambda **kw: stack.enter_context(tc.tile_pool(**kw))

    singles = pool(name="singles", bufs=1)
    big = pool(name="big", bufs=1)
    stage = pool(name="stage", bufs=3)
    d16p = pool(name="d16p", bufs=3)
    psum = pool(name="psum", bufs=1, space="PSUM")
    p_pool = pool(name="p_pool", bufs=2)
    pt_pool = pool(name="pt_pool", bufs=3)
    stats = pool(name="stats", bufs=3)
    osb = pool(name="osb", bufs=2)

    with stack:
        ident = singles.tile([P, P], F16)
        make_identity(nc, ident[:])
        dummy = singles.tile([P, KC], F16)
        nc.vector.memset(dummy[:], 0.25)

        v_flat = big.tile([P, seq * d // P], F16)  # enc f16, slot j = [:, j*d:...]
        kT = big.tile([P, n_dt, seq], F16)         # enc^T  [d_in, dt, kfree]
        qT = big.tile([P, n_qt, n_dt, P], F16)     # dec^T  [d_in, j, dt, q]

        def warm_burst(n, name):
            wp = psum.tile([P, KC], F32, tag="po", bufs=4, name=name)
            for i in range(n):
                nc.tensor.matmul(
                    wp[:], dummy[:, 0:P], dummy[:], start=(i == 0), stop=(i == n - 1)
                )

        warm_burst(12, "warm0")

        # ---- input loads (fat descriptors) ----
        EQ = 2048  # enc quarter (f32 elems per partition, covers 4 j-slots)
        DC = 1024  # dec chunk (2 j-slots)
        enc32 = {}
        dec32 = {}

        def load(eng, src, width, lo, name):
            t = stage.tile([P, width], F32, tag="ld32", bufs=6, name=name)
            eng.dma_start(out=t[:], in_=src[:, lo : lo + width])
            return t

        # sync: enc E0, dec c0, enc E1, dec c1..c7; scalar: enc E2, E3
        enc32[0] = load(nc.sync, enc_flat, EQ, 0, "e32_0")
        enc32[2] = load(nc.scalar, enc_flat, EQ, 2 * EQ, "e32_2")
        dec32[0] = load(nc.sync, dec_flat, DC, 0, "d32_0")
        enc32[3] = load(nc.scalar, enc_flat, EQ, 3 * EQ, "e32_3")
        enc32[1] = load(nc.sync, enc_flat, EQ, EQ, "e32_1")
        for c in range(1, 8):
            dec32[c] = load(nc.sync, dec_flat, DC, c * DC, f"d32_{c}")

        # ---- enc^T via PE transposes in 2-tile pair groups; pair copy
        # [128,8,128] PSUM->SBUF on DVE (j0-7) / ScalarE (j8-15)
        def prep_kT_pair(jp, eng):
            tps = psum.tile(
                [P, n_dt, 2, P], F16, tag="po", bufs=4, name=f"tps_{jp}"
            )
            for hh in range(2):
                j = 2 * jp + hh
                for dc in range(n_dt):
                    nc.tensor.transpose(
                        tps[:, dc, hh, :],
                        v_flat[:, j * d + dc * P : j * d + (dc + 1) * P],
                        ident[:],
                    )
            dst = kT[:, :, 2 * jp * P : (2 * jp + 2) * P].rearrange(
                "p dt (hh b) -> p dt hh b", hh=2
            )
            if eng is nc.vector:
                eng.tensor_copy(dst, tps[:])
            else:
                eng.activation(dst, tps[:], AFT.Copy)

        def cast_enc(e):
            dst = v_flat[:, e * EQ : (e + 1) * EQ]
            if e in (0, 1):
                nc.vector.tensor_copy(dst, enc32[e][:])
            else:
                nc.scalar.activation(dst, enc32[e][:], AFT.Copy)

        # ---- dec casts (DVE); qT tiles 0-3 via PE transposes in the
        # prologue, later tiles ride the xbar chain one per iteration
        d16 = {}

        def cast_dec(c):
            t = d16p.tile([P, DC], F16, tag="d16", name=f"d16_{c}")
            nc.gpsimd.tensor_copy(t[:], dec32[c][:])
            d16[c] = t

        prev_t = [None]

        def xbar(out_tile, in_view, eng=None):
            ti = (eng or nc.scalar).dma_start(
                out=out_tile, in_=in_view, transpose=True
            )
            if prev_t[0] is not None:
                tile.add_dep_helper(ti.ins, prev_t[0].ins, reason="serialize xbar")
            prev_t[0] = ti

        def prep_qT_pe(j):
            c, h = j // 2, j % 2
            tps = psum.tile([P, n_dt, P], F16, tag="po", bufs=4, name=f"tq_{j}")
            for dc in range(n_dt):
                nc.tensor.transpose(
                    tps[:, dc, :], d16[c][:, h * d + dc * P : h * d + (dc + 1) * P],
                    ident[:],
                )
            nc.vector.tensor_copy(qT[:, j, :, :], tps[:])

        cast_enc(0)          # DVE, j0-3
        cast_enc(2)          # ScalarE, j8-11
        prep_kT_pair(0, nc.vector)
        prep_kT_pair(1, nc.vector)
        cast_dec(0)          # gpsimd
        prep_qT_pe(0)
        prep_qT_pe(1)
        prep_kT_pair(4, nc.scalar)
        prep_kT_pair(5, nc.scalar)
        cast_enc(1)          # DVE, j4-7
        cast_enc(3)          # ScalarE, j12-15
        prep_kT_pair(2, nc.vector)
        prep_kT_pair(3, nc.vector)
        prep_kT_pair(6, nc.vector)
        prep_kT_pair(7, nc.vector)

        # ---- main loop ----
        pending = [None]
        PACE_W = 0  # f16 elems; tunes the per-tile PE idle gap (DVFS duty)
        pace_sb = (
            singles.tile([P, PACE_W], F16, name="pace_sb") if PACE_W else None
        )

        def emit_O(st, pacer):
            qt, pt, g = st
            o_cs = []
            for c in range(n_sc):
                o_c = psum.tile([P, d], F32, tag="po", bufs=4, name=f"o_{qt}_{c}")
                for jj in range(jps):
                    mi = nc.tensor.matmul(
                        o_c[:],
                        pt[:, c * jps + jj, :],
                        v_flat[:, (c * jps + jj) * d : (c * jps + jj + 1) * d],
                        start=(jj == 0),
                        stop=(jj == jps - 1),
                    )
                    if c == 0 and jj == 0 and pacer is not None:
                        tile.add_dep_helper(mi.ins, pacer.ins, reason="pace PE")
                o_cs.append(o_c)
            o_s0 = osb.tile([P, d], F32, tag="osb0", name=f"osb0_{qt}")
            o_s1 = osb.tile([P, d], F32, tag="osb1", name=f"osb1_{qt}")
            nc.vector.tensor_scalar_mul(o_s0[:], o_cs[0][:], g[:, 0:1])
            nc.vector.tensor_scalar_mul(o_s1[:], o_cs[1][:], g[:, 1:2])
            o_f = osb.tile([P, d], F32, tag="osbf", name=f"osbf_{qt}")
            nc.gpsimd.tensor_tensor(
                o_f[:], o_s0[:], o_s1[:], op=mybir.AluOpType.add
            )
            nc.sync.dma_start(out=out_view[:, qt, :], in_=o_f[:])

        for qt in range(n_qt):
            q0 = qt * P
            # S matmuls chunk-major; partial row-max reduces interleaved
            nmx4 = stats.tile([P, n_sc * n_hf], F32, tag="nmx4")
            s_cs = []
            for c in range(n_sc):
                s_c = psum.tile([P, SC], F32, tag="s", bufs=2, name=f"s_{qt}_{c}")
                for h in range(n_hf):
                    k0 = c * SC + h * KC
                    for dt_ in range(n_dt):
                        nc.tensor.matmul(
                            s_c[:, h * KC : (h + 1) * KC],
                            qT[:, qt, dt_, :],
                            kT[:, dt_, k0 : k0 + KC],
                            start=(dt_ == 0),
                            stop=(dt_ == n_dt - 1),
                        )
                    nc.vector.tensor_reduce(
                        nmx4[:, c * n_hf + h : c * n_hf + h + 1],
                        s_c[:, h * KC : (h + 1) * KC],
                        axis=AX.X, op=mybir.AluOpType.max, negate=True,
                    )
                s_cs.append(s_c)

            # per-chunk softmax (exp with own max) + one full-row P^T
            nmx = stats.tile([P, n_sc], F32, tag="nmx")
            sums = stats.tile([P, n_sc], F32, tag="sums")
            p_sb = p_pool.tile([P, seq], F16)
            pacer = None
            for c in range(n_sc):
                nc.vector.tensor_reduce(
                    nmx[:, c : c + 1], nmx4[:, c * n_hf : (c + 1) * n_hf],
                    axis=AX.X, op=mybir.AluOpType.min,
                )
                if c == n_sc - 1 and pace_sb is not None:
                    pacer = nc.vector.memset(pace_sb[:], 0.5)
                nc.scalar.activation(
                    p_sb[:, c * SC : (c + 1) * SC],
                    s_cs[c][:],
                    AFT.Exp,
                    bias=nmx[:, c : c + 1],
                    scale=1.0,
                    accum_out=sums[:, c : c + 1],
                )
            pt = pt_pool.tile([P, seq // P, P], F16, tag="pt", name=f"pt_{qt}")
            if qt < n_qt - 1:
                xbar(pt[:], p_sb[:])

            # global correction: g_c = exp(m_c - m_g) / Z
            mxp = stats.tile([P, n_sc], F32, tag="mxp")
            nc.vector.tensor_scalar_mul(mxp[:], nmx[:], -1.0)
            negmg = stats.tile([P, 1], F32, tag="negmg")
            nc.vector.tensor_reduce(
                negmg[:], nmx4[:], axis=AX.X, op=mybir.AluOpType.min
            )
            fsc = stats.tile([P, n_sc], F32, tag="fsc")
            nc.scalar.activation(fsc[:], mxp[:], AFT.Exp, bias=negmg[:], scale=1.0)
            ssc = stats.tile([P, n_sc], F32, tag="ssc")
            nc.vector.tensor_mul(ssc[:], sums[:], fsc[:])
            sm = stats.tile([P, 1], F32, tag="sm")
            nc.vector.reduce_sum(sm[:], ssc[:], axis=AX.X)
            rinv = stats.tile([P, 1], F32, tag="rinv")
            nc.vector.reciprocal(rinv[:], sm[:])
            g = stats.tile([P, n_sc], F32, tag="g")
            nc.vector.tensor_scalar_mul(g[:], fsc[:], rinv[:])

            if pending[0] is not None:
                emit_O(pending[0], pacer)
            if qt == n_qt - 1:
                # last tile: P^T on the now-idle PE (the xbar chain latency
                # plus the tail downclock would cost ~5us here)
                for grp in range(4):
                    tq = psum.tile(
                        [P, 4, P], F16, tag="po", bufs=4, name=f"ptpe_{grp}"
                    )
                    for k2 in range(4):
                        j = grp * 4 + k2
                        nc.tensor.transpose(
                            tq[:, k2, :], p_sb[:, j * P : (j + 1) * P], ident[:]
                        )
                    nc.vector.tensor_copy(pt[:, grp * 4 : (grp + 1) * 4, :], tq[:])
            pending[0] = (qt, pt, g)

            # dec chunk casts at DVE tail (every other iter); qT for tile
            # qt+4 via PE transposes at the block tail (~0.45us, keeps PE
            # duty under the DVFS activity threshold)
            if qt == 0:
                cast_dec(1)
                prep_qT_pe(2)
                prep_qT_pe(3)
            if qt % 2 == 0 and qt // 2 + 2 < (seq * d // P) // DC:
                cast_dec(qt // 2 + 2)
            if qt + 4 < n_qt:
                prep_qT_pe(qt + 4)

        emit_O(pending[0], None)


def build(seq=2048, d=512, n_cores=N_CORES):
    nc = bacc.Bacc(
        "TRN2", target_bir_lowering=False, debug=False, num_devices=n_cores
    )
    dec = nc.dram_tensor("dec", [seq, d], F32, kind="ExternalInput").ap()
    enc = nc.dram_tensor("enc", [seq, d], F32, kind="ExternalInput").ap()
    out = nc.dram_tensor("out", [seq, d], F32, kind="ExternalOutput").ap()
    with tile.TileContext(nc) as tc:
        attention_tile_kernel(tc, out, dec, enc, seq, d)
    nc.compile()
    return nc


# ---------------------------------------------------------------------------
# Optional NTFF profiling support (used by our own test harness; inert unless
# BASSKERNEL_TRACE=1). The agent image lacks `antenv.axon_hooks`, so recreate
# it in sys.modules with a ctypes hook against libaxon_pjrt.so.
# ---------------------------------------------------------------------------
LAST_EXEC_TIME_NS = None


def _install_profile_hook():
    so_path = "/opt/axon/libaxon_pjrt.so"
    if "antenv.axon_hooks" in sys.modules or not os.path.exists(so_path):
        return
    lib = ctypes.CDLL(so_path)
    if not hasattr(lib, "axon_start_nrt_profile"):
        return
    lib.axon_start_nrt_profile.argtypes = [
        ctypes.POINTER(ctypes.c_int64),
        ctypes.c_size_t,
    ]
    lib.axon_start_nrt_profile.restype = ctypes.c_int64
    lib.axon_stop_nrt_profile.argtypes = [ctypes.c_char_p]
    lib.axon_stop_nrt_profile.restype = ctypes.c_int64

    @contextlib.contextmanager
    def _hook(output_dir, device_ids):
        import jax

        jax.devices()
        if device_ids:
            ids = (ctypes.c_int64 * len(device_ids))(*device_ids)
            rc = lib.axon_start_nrt_profile(ids, len(device_ids))
        else:
            rc = lib.axon_start_nrt_profile(None, 0)
        if rc != 0:
            raise RuntimeError(f"axon_start_nrt_profile rc={rc}")
        try:
            yield
        finally:
            n = lib.axon_stop_nrt_profile(str(output_dir).encode())
            print(f"ntff profile: {n} file(s) written to {output_dir}")

    mod = types.ModuleType("antenv.axon_hooks")
    _state = {"hook": _hook}
    mod.set_axon_ntff_profile_hook = lambda h: _state.__setitem__("hook", h)
    mod.get_axon_ntff_profile_hook = lambda: _state["hook"]
    sys.modules["antenv.axon_hooks"] = mod
    bass_utils.upload_artifacts = lambda tmpdir: tmpdir


_NC_CACHE = {}


def kernel(enc_outputs: np.ndarray, dec_outputs: np.ndarray) -> np.ndarray:
    B, seq, d = dec_outputs.shape
    assert enc_outputs.shape == (B, seq, d) and B == N_CORES

    trace = os.environ.get("BASSKERNEL_TRACE", "0") == "1"
    if trace:
        _install_profile_hook()

    key = (seq, d)
    if key not in _NC_CACHE:
        _NC_CACHE[key] = build(seq, d)
    nc = _NC_CACHE[key]

    in_maps = [
        {
            "dec": np.ascontiguousarray(dec_outputs[b], dtype=np.float32),
            "enc": np.ascontiguousarray(enc_outputs[b], dtype=np.float32),
        }
        for b in range(B)
    ]
    res = bass_utils.run_bass_kernel_spmd(
        nc,
        in_maps,
        core_ids=list(range(N_CORES)),
        trace=trace,
        tmpdir=os.environ.get("BASSKERNEL_TRACE_DIR") if trace else None,
    )
    global LAST_EXEC_TIME_NS
    LAST_EXEC_TIME_NS = res.exec_time_ns
    out = np.stack([res.results[b]["out"] for b in range(B)], axis=0)
    return out.astype(np.float32)
